# revision 3
# baseline (speedup 1.0000x reference)
"""Trainium2 Bass kernel for LongcatFlash top-k router.

Contract: kernel(**inputs) takes the FULL unsharded inputs
(hidden_states [8192, 6144] f32, classifier_weight [768, 6144] f32,
e_score_correction_bias [768] f32) and returns
(topk_indices int32 [8192, 12], topk_weights f32 [8192, 12]) matching
reference semantics:

    logits = x @ w.T                       (fp32)
    s      = softmax(logits, axis=-1)
    sfc    = s + bias
    idx    = top_k(sfc, 12).indices
    wts    = s[idx] * 2.5

Sharding: data-parallel over 8 NeuronCores — 1024 tokens per core,
router weight + bias replicated.

Device pipeline per core (8 token-tiles of 128 tokens):
  - matmul in bf16 hi/lo 3-pass (x_hi@w_hi + x_hi@w_lo + x_lo@w_hi) with
    fp32 PSUM accumulation: logit error ~2e-6, verified to reproduce the
    fp32 reference's expert selection exactly on the problem distribution.
  - softmax via ScalarE exp (PSUM -> SBUF, fused row-sum accumulator),
    DVE reciprocal. The max-subtraction is skipped: logits ~ N(0,1), so
    exp() stays comfortably in fp32 range and ACT exp is <=2 ULP.
  - top-12 of 2.5*(s + b) via DVE max8 / max_index / match_replace
    (two rounds). Scaling by 2.5 > 0 preserves the selection order and
    folds the final weight scale for free.
  - exact weights on device: a third match_replace marks the 12 selected
    positions; their (2.5*b) values are extracted with a second max8 pass
    over a bias tensor masked to the selected positions, then re-associated
    to the selection order with 12 tiny 16-wide compare+reduce ops.
    weights = sel_value - 2.5*b[idx] = 2.5*s[idx], bit-accurate gather.
"""

import numpy as np
import ml_dtypes

import concourse.bacc as bacc
import concourse.mybir as mybir
from concourse.tile import TileContext

TOKENS = 8192
HIDDEN = 6144
E = 768
TOPK = 12
SCALE = 2.5
N_CORES = 8
P = 128
TPC = TOKENS // N_CORES          # tokens per core = 1024
TT = TPC // P                    # token tiles per core = 8
KT = HIDDEN // P                 # contraction tiles = 48
NEG_BIG = -1.0e30
BAD_POS = 1_000_000_000          # sanitizer for unused max_index slots

_CACHE: dict = {}


def _build_nc():
    f32 = mybir.dt.float32
    bf16 = mybir.dt.bfloat16
    u32 = mybir.dt.uint32
    Alu = mybir.AluOpType

    nc = bacc.Bacc(None, target_bir_lowering=False)

    xh_d = nc.dram_tensor("xh", [TT, P, HIDDEN], bf16, kind="ExternalInput")
    xl_d = nc.dram_tensor("xl", [TT, P, HIDDEN], bf16, kind="ExternalInput")
    wh_d = nc.dram_tensor("wh", [P, KT, E], bf16, kind="ExternalInput")
    wl_d = nc.dram_tensor("wl", [P, KT, E], bf16, kind="ExternalInput")
    b25_d = nc.dram_tensor("b25", [P, E], f32, kind="ExternalInput")
    oidx_d = nc.dram_tensor("oidx", [TT, P, TOPK], u32, kind="ExternalOutput")
    owts_d = nc.dram_tensor("owts", [TT, P, TOPK], f32, kind="ExternalOutput")

    with TileContext(nc) as tc:
        with (
            tc.tile_pool(name="wpool", bufs=1) as wpool,
            tc.tile_pool(name="xpool", bufs=3) as xpool,
            tc.tile_pool(name="epool", bufs=1) as epool,
            tc.tile_pool(name="spool", bufs=2) as spool,
            tc.tile_pool(name="cpool", bufs=1) as cpool,
            tc.tile_pool(name="ppool", bufs=4, space="PSUM") as ppool,
        ):
            # --- resident weights, one tile per k so matmuls only wait on
            # the k-slices they actually read (overlaps the 19MB preload).
            wh_t = []
            wl_t = []
            for k in range(KT):
                th = wpool.tile([P, E], bf16, name=f"wh{k}")
                nc.sync.dma_start(th, wh_d[:, k, :])
                wh_t.append(th)
            for k in range(KT):
                tl = wpool.tile([P, E], bf16, name=f"wl{k}")
                nc.sync.dma_start(tl, wl_d[:, k, :])
                wl_t.append(tl)

            b25_t = cpool.tile([P, E], f32, name="b25c")
            nc.sync.dma_start(b25_t, b25_d[:, :])
            negbig_t = cpool.tile([P, E], f32, name="negbig")
            nc.vector.memset(negbig_t, NEG_BIG)

            for tt in range(TT):
                # ---------------- matmul: logits'[t, e] ----------------
                xh_t = xpool.tile([P, KT, P], bf16, tag="xtile", name=f"xh{tt}")
                nc.sync.dma_start(xh_t, xh_d[tt].rearrange("p (k t) -> p k t", k=KT))
                pt0 = ppool.tile([P, 512], f32, tag="pt0", name=f"pt0_{tt}")
                pt1 = ppool.tile([P, 256], f32, tag="pt1", name=f"pt1_{tt}")
                for k in range(KT):
                    lhs = xh_t[:, k, :]
                    nc.tensor.matmul(pt0, lhs, wh_t[k][:, 0:512],
                                     start=(k == 0), stop=False)
                    nc.tensor.matmul(pt1, lhs, wh_t[k][:, 512:768],
                                     start=(k == 0), stop=False)
                    nc.tensor.matmul(pt0, lhs, wl_t[k][:, 0:512],
                                     start=False, stop=False)
                    nc.tensor.matmul(pt1, lhs, wl_t[k][:, 512:768],
                                     start=False, stop=False)
                xl_t = xpool.tile([P, KT, P], bf16, tag="xtile", name=f"xl{tt}")
                nc.sync.dma_start(xl_t, xl_d[tt].rearrange("p (k t) -> p k t", k=KT))
                for k in range(KT):
                    lhs = xl_t[:, k, :]
                    nc.tensor.matmul(pt0, lhs, wh_t[k][:, 0:512],
                                     start=False, stop=(k == KT - 1))
                    nc.tensor.matmul(pt1, lhs, wh_t[k][:, 512:768],
                                     start=False, stop=(k == KT - 1))

                # ---------------- softmax', sfc' = 2.5*(s + b) ----------------
                exp_t = epool.tile([P, E], f32, tag="exp", name=f"exp{tt}")
                z0 = spool.tile([P, 1], f32, name=f"z0_{tt}")
                z1 = spool.tile([P, 1], f32, name=f"z1_{tt}")
                nc.scalar.activation(exp_t[:, 0:512], pt0,
                                     mybir.ActivationFunctionType.Exp,
                                     accum_out=z0)
                nc.scalar.activation(exp_t[:, 512:768], pt1,
                                     mybir.ActivationFunctionType.Exp,
                                     accum_out=z1)
                rz = spool.tile([P, 1], f32, name=f"rz{tt}")
                zs = spool.tile([P, 1], f32, name=f"zs{tt}")
                nc.vector.tensor_add(zs, z0, z1)
                nc.vector.reciprocal(rz, zs)
                nc.vector.tensor_scalar_mul(rz, rz, SCALE)  # rz = 2.5/Z
                sfc = epool.tile([P, E], f32, tag="sfc", name=f"sfc{tt}")
                # sfc' = exp * (2.5/Z) + 2.5*b
                nc.vector.scalar_tensor_tensor(sfc, exp_t, rz, b25_t,
                                               Alu.mult, Alu.add)

                # ---------------- top-12 selection ----------------
                mst = spool.tile([P, 16], f32, name=f"mst{tt}")
                idxs = spool.tile([P, 16], u32, name=f"idx{tt}")
                sfc2 = epool.tile([P, E], f32, tag="sfc2", name=f"sfc2_{tt}")
                nc.vector.max(mst[:, 0:8], sfc)
                nc.vector.max_index(idxs[:, 0:8], mst[:, 0:8], sfc)
                nc.vector.match_replace(sfc2, mst[:, 0:8], sfc, NEG_BIG)
                nc.vector.max(mst[:, 8:16], sfc2)
                nc.vector.max_index(idxs[:, 8:16], mst[:, 8:16], sfc2)

                # mark ranks 8..11 too: replace first 4 of round-2 maxima
                m2p = spool.tile([P, 8], f32, name=f"m2p{tt}")
                nc.vector.tensor_copy(m2p[:, 0:4], mst[:, 8:12])
                nc.vector.memset(m2p[:, 4:8], NEG_BIG)
                nc.vector.match_replace(sfc2, m2p, sfc2, NEG_BIG)
                # mask of the 12 selected expert positions (int dtype —
                # copy_predicated requires an integer mask)
                mask = epool.tile([P, E], mybir.dt.uint8, tag="mask",
                                  name=f"mask{tt}")
                nc.vector.tensor_tensor(mask, sfc2, sfc, Alu.is_lt)

                # ---------------- exact bias gather ----------------
                # tb = 2.5*b at selected positions, -BIG elsewhere
                tb = epool.tile([P, E], f32, tag="exp", name=f"tb{tt}")
                nc.vector.tensor_copy(tb, negbig_t)
                nc.vector.copy_predicated(tb, mask, b25_t)
                bvals = spool.tile([P, 16], f32, name=f"bv{tt}")
                bpos = spool.tile([P, 16], u32, name=f"bp{tt}")
                nc.vector.max(bvals[:, 0:8], tb)
                nc.vector.max_index(bpos[:, 0:8], bvals[:, 0:8], tb)
                nc.vector.match_replace(tb, bvals[:, 0:8], tb, NEG_BIG)
                nc.vector.max(bvals[:, 8:16], tb)
                nc.vector.max_index(bpos[:, 8:16], bvals[:, 8:16], tb)
                # slots 12..15 hold garbage positions of -BIG values: poison
                nc.vector.memset(bpos[:, 12:16], BAD_POS)

                # associate: b25sel[j] = bvals[i] where bpos[i] == idxs[j].
                # One fused native DVE op per j:
                #   m16 = (bpos == idx_j) * bvals ; accum = sum(m16)
                b25sel = spool.tile([P, TOPK], f32, name=f"bsel{tt}")
                m16 = spool.tile([P, 16], f32, name=f"m16_{tt}")
                for j in range(TOPK):
                    nc.vector.scalar_tensor_tensor(
                        m16, bpos, idxs[:, j:j + 1], bvals,
                        Alu.is_equal, Alu.mult,
                        accum_out=b25sel[:, j:j + 1])

                # weights = sfc'[idx] - 2.5*b[idx] = 2.5 * s[idx]
                w12 = spool.tile([P, TOPK], f32, name=f"w12_{tt}")
                nc.vector.tensor_sub(w12, mst[:, 0:TOPK], b25sel)

                nc.sync.dma_start(oidx_d[tt], idxs[:, 0:TOPK])
                nc.sync.dma_start(owts_d[tt], w12)

    nc.finalize()
    return nc


def _prep_inputs(hidden_states, classifier_weight, e_score_correction_bias):
    bf16 = ml_dtypes.bfloat16
    x = np.ascontiguousarray(np.asarray(hidden_states, dtype=np.float32))
    w = np.ascontiguousarray(np.asarray(classifier_weight, dtype=np.float32))
    b = np.asarray(e_score_correction_bias, dtype=np.float32)

    xh = x.astype(bf16)
    xl = (x - xh.astype(np.float32)).astype(bf16)
    wh = w.astype(bf16)
    wl = (w - wh.astype(np.float32)).astype(bf16)

    # w tiles: [p, k, e] with p = h % 128, k = h // 128
    def wtile(a):  # [768, 6144] -> [128, 48, 768]
        return np.ascontiguousarray(a.reshape(E, KT, P).transpose(2, 1, 0))

    wh_t = wtile(wh)
    wl_t = wtile(wl)
    b25 = np.ascontiguousarray(
        np.broadcast_to((SCALE * b.astype(np.float64)).astype(np.float32)[None, :],
                        (P, E)))

    # x tiles per core: [tt, p, k*128 + t]
    def xtile(a_core):  # [1024, 6144] -> [8, 128, 6144]
        return np.ascontiguousarray(
            a_core.reshape(TT, P, KT, P).transpose(0, 3, 2, 1).reshape(TT, P, HIDDEN))

    in_maps = []
    for c in range(N_CORES):
        sl = slice(c * TPC, (c + 1) * TPC)
        in_maps.append({
            "xh": xtile(xh[sl]),
            "xl": xtile(xl[sl]),
            "wh": wh_t,
            "wl": wl_t,
            "b25": b25,
        })
    return in_maps


def _get_runner():
    """Build + compile once per process; reuse the jitted executable."""
    if "runner" in _CACHE:
        return _CACHE["runner"]
    from concourse.bass_utils import run_bass_kernel_spmd

    nc = _build_nc()

    def runner(in_maps):
        res = run_bass_kernel_spmd(nc, in_maps, core_ids=list(range(N_CORES)))
        return res.results

    _CACHE["runner"] = runner
    return runner


def kernel(hidden_states, classifier_weight, e_score_correction_bias):
    in_maps = _prep_inputs(hidden_states, classifier_weight,
                           e_score_correction_bias)
    results = _get_runner()(in_maps)

    idx = np.concatenate(
        [r["oidx"].reshape(TPC, TOPK) for r in results], axis=0).astype(np.int32)
    wts = np.concatenate(
        [r["owts"].reshape(TPC, TOPK) for r in results], axis=0).astype(np.float32)
    return idx, wts


if __name__ == "__main__":
    rng = np.random.default_rng(0)
    x = rng.standard_normal((TOKENS, HIDDEN), dtype=np.float32)
    w = rng.standard_normal((E, HIDDEN), dtype=np.float32) / np.sqrt(HIDDEN)
    b = (rng.standard_normal(E) * 0.1).astype(np.float32)
    idx, wts = kernel(x, w, b)
    print(idx.shape, wts.shape, idx.dtype, wts.dtype)
